# revision 1
# baseline (speedup 1.0000x reference)
"""Multi-head attention (B=2, S=2048, DIM=512, H=8) on 8 Trainium2 cores.

Sharding: data-parallel over batch x tensor-parallel over heads.
Core c handles batch b = c // 4 and heads {2g, 2g+1} where g = c % 4
(i.e. output feature columns [128g : 128g+128]).  All sharding /
gathering happens host-side; no on-device collectives.

Per-core kernel.  The softmax exp is the binding resource: 8.4M
elements/core through ScalarE's 1 elem/lane/cycle @1.2GHz datapath is
a ~55us floor (72.6us at the [128,1024] instruction width PSUM
allows), so the whole schedule is built to keep ScalarE's exp train
saturated from ~15us to ~92us.

Key structural choices:
  - q/k projection path in fp8-e3m4 (inputs AND weights; weights
    host-scaled by 16 into fp8's normal range, the 1/16^2 folded into
    the exp's fused scale) - halves the critical head DMA; the v path
    stays fp16 so the ctx output keeps ~1.2e-2 rel err,
  - inputs host-prearranged so each 512-seq chunk is one
    partition-contiguous tile; q/k chunks striped across the two
    HWDGE queues (all issued upfront), v bulk on the gpsimd SWDGE
    queue; ScalarE's queue carries no input DMAs once exps start,
  - 16 dummy warmup matmuls during the first chunk's DMA wait so the
    PE's HAM clock gate is at 2.4GHz when real work arrives,
  - wavefront schedule: q1's first 12 score tiles (which only need
    QT chunk 1 + KT tile t) interleave into the PE-heavy q0
    projection phase; every q-boundary pre-feeds the next block's
    scores so ScalarE's exp queue never drains (the sps double
    buffer caps the backlog at ~2.3us, so score emission is batched,
    never fine-interleaved - the PE is in-order and a score matmul
    stalled on the sps WAR would block ctx work behind it),
  - ctx-PSUM evacuation on DVE at mid-kernel boundaries (ScalarE is
    the saturated engine; only the final tail borrows it); output
    DRAM tensor is [128, S/128*128] (seq tiled over partitions) so
    each q-block stores with one contiguous DMA; host untiles.

Compute structure:
  - Q^T, K^T projections in [out_dim(128), seq] layout (head h at
    partitions 64h..64h+63) - attention-ready; V in natural
    [seq, out_dim] tiles with a ones column per head so the ctx
    matmul also accumulates the softmax denominator for free,
  - scores^T = K_h @ Q_h^T per 128-row key tile (K=64), the two
    heads' matmuls target disjoint PE row groups and run
    concurrently; exp on ScalarE with the 1/sqrt(512) scale fused
    ([128,1024] tiles, fp16 output),
  - ctx^T accumulated over key tiles (lhsT = V tile [128,65],
    rhs = exp-scores [128,512], fp32 PSUM),
  - PE transpose back to natural layout, reciprocal + scale, DMA out.
"""

import os

import ml_dtypes
import numpy as np

DIM = 512
NUM_HEADS = 8
D_HEAD = 64
B = 2
S = 2048
N_CORES = 8
P = 128  # partitions
NK = DIM // P  # 4 contraction tiles for projections
NT = S // P  # 16 key tiles
VSTRIDE = 132  # V tile stride: [h0(64) | ones | h1(64) | 3 pad]
CH = 512  # input DMA / projection chunk (columns of seq)
# q/k path runs in fp8-e3m4: host scales Wq/Wk (and bq/bk) by WSCALE so
# the weights sit in fp8's normal range; scores come out scaled by
# WSCALE^2, folded into the exp's fused scale.
WSCALE = 16.0
SCALE = float(1.0 / np.sqrt(512.0) / (WSCALE * WSCALE))

_CACHE = {}


def _build_program():
    import concourse.tile as tile
    from concourse import bacc, mybir

    f32 = mybir.dt.float32
    f16 = mybir.dt.float16
    f8 = mybir.dt.float8e3
    nc = bacc.Bacc("TRN2", target_bir_lowering=False, debug=False)

    io = {}
    # [p, c*2048 + k*512 + s] = x[c*512+s, 128k+p]
    io["xq"] = nc.dram_tensor("xq", [P, NK * S], f8, kind="ExternalInput").ap()
    io["xk"] = nc.dram_tensor("xk", [P, NK * S], f8, kind="ExternalInput").ap()
    io["xv"] = nc.dram_tensor("xv", [P, NK * S], f16, kind="ExternalInput").ap()
    io["wq"] = nc.dram_tensor("wq", [P, DIM], f8, kind="ExternalInput").ap()
    io["wk"] = nc.dram_tensor("wk", [P, DIM], f8, kind="ExternalInput").ap()
    io["wv"] = nc.dram_tensor("wv", [P, DIM], f16, kind="ExternalInput").ap()
    io["bq2"] = nc.dram_tensor("bq2", [P, 1], f32, kind="ExternalInput").ap()
    io["bk2"] = nc.dram_tensor("bk2", [P, 1], f32, kind="ExternalInput").ap()
    io["bvb"] = nc.dram_tensor("bvb", [P, P], f32, kind="ExternalInput").ap()
    io["ident"] = nc.dram_tensor("ident", [P, P], f32, kind="ExternalInput").ap()
    # [p, t*128 + d] = out_natural[t*128 + p, d]
    io["out"] = nc.dram_tensor("out", [P, NT * P], f32, kind="ExternalOutput").ap()

    with tile.TileContext(nc) as tc:
        _emit(tc, mybir, io)
    nc.compile()
    return nc


def _emit(tc, mybir, io):
    from contextlib import ExitStack

    nc = tc.nc
    f32 = mybir.dt.float32
    f16 = mybir.dt.float16
    f8 = mybir.dt.float8e3
    Exp = mybir.ActivationFunctionType.Exp

    mm = nc.tensor.matmul

    with ExitStack() as ctx:
        const = ctx.enter_context(tc.tile_pool(name="const", bufs=1))
        qk = ctx.enter_context(tc.tile_pool(name="qk", bufs=1))
        vpool = ctx.enter_context(tc.tile_pool(name="vpool", bufs=1))
        csbpool = ctx.enter_context(tc.tile_pool(name="csbp", bufs=2))
        opool = ctx.enter_context(tc.tile_pool(name="opool", bufs=2))

        # constants, split across the three DMA queues so they don't
        # delay the input stream behind them
        wq_sb = const.tile([P, DIM], f8, tag="wq")
        wk_sb = const.tile([P, DIM], f8, tag="wk")
        wv_sb = const.tile([P, DIM], f16, tag="wv")
        bq_sb = const.tile([P, 1], f32, tag="bq")
        bk_sb = const.tile([P, 1], f32, tag="bk")
        bvb_sb = const.tile([P, P], f32, tag="bvb")
        id_sb = const.tile([P, P], f32, tag="ident")
        nc.sync.dma_start(wq_sb[:], io["wq"][:])
        nc.scalar.dma_start(wk_sb[:], io["wk"][:])
        nc.gpsimd.dma_start(bq_sb[:], io["bq2"][:])
        nc.gpsimd.dma_start(bk_sb[:], io["bk2"][:])
        nc.gpsimd.dma_start(wv_sb[:], io["wv"][:])
        nc.gpsimd.dma_start(bvb_sb[:], io["bvb"][:])
        nc.gpsimd.dma_start(id_sb[:], io["ident"][:])

        # persistent projection outputs
        QT = qk.tile([P, S], f16, tag="QT")  # [out_dim, seq]
        KT = qk.tile([P, S], f16, tag="KT")
        V = vpool.tile([P, NT * VSTRIDE], f16, tag="V")  # 16 x [128, 132]

        with (
            tc.tile_pool(name="xin", bufs=4) as xin,
            tc.tile_pool(name="psq", bufs=2, space="PSUM") as psq,
            tc.tile_pool(name="pss", bufs=2, space="PSUM") as pss,
            tc.tile_pool(name="psc", bufs=1, space="PSUM") as psc,
            tc.tile_pool(name="es", bufs=14) as espool,
        ):
            # ones columns of V (shared between the two heads of each
            # key tile), one strided memset
            nc.vector.memset(
                V[:].rearrange("p (t c) -> p t c", c=VSTRIDE)[:, :, 64:65], 1.0
            )

            def load_chunk(c):
                # q/k chunks striped half/half across the two HWDGE
                # queues (all issued upfront, so the scalar engine's
                # issue instructions finish before the exp train
                # starts); v bulk on the gpsimd SWDGE queue.
                w = NK * CH
                cs = slice(c * w, (c + 1) * w)
                xt = {}
                for name, key in (("q", "xq"), ("k", "xk")):
                    tl = xin.tile([P, w], f8, tag="x" + name, name="xt")
                    h = w // 2
                    nc.sync.dma_start(tl[:, 0:h], io[key][:, c * w : c * w + h])
                    nc.scalar.dma_start(
                        tl[:, h:w], io[key][:, c * w + h : (c + 1) * w]
                    )
                    xt[name] = tl
                tl = xin.tile([P, w], f16, tag="xv", name="xt")
                nc.gpsimd.dma_start(tl[:], io["xv"][:, cs])
                xt["v"] = tl
                return xt

            def pe_warmup(n):
                # dummy matmuls into a scratch PSUM tile while the
                # first input chunk streams in: keeps the PE's HAM
                # activity window busy so the real projections run at
                # 2.4GHz instead of the cold 1.2GHz
                ps = psq.tile([P, CH], f32, tag="psq", name="warm")
                for i in range(n):
                    mm(ps[:], wq_sb[:, 0:P], wq_sb[:], start=True, stop=True)

            def proj_qk(c, xt):
                # k first with its bias-add on ScalarE for chunk 0 (the
                # critical path to the first exp), q's add on DVE
                # overlapping the k matmuls
                cs = slice(c * CH, (c + 1) * CH)
                def piece(w_sb, b_sb, dst, src, lo, hi):
                    ps = psq.tile([P, CH], f32, tag="psq", name="psq")
                    for k in range(NK):
                        mm(
                            ps[:, 0 : hi - lo],
                            w_sb[:, k * P : (k + 1) * P],
                            src[:, k * CH + lo : k * CH + hi],
                            start=(k == 0),
                            stop=(k == NK - 1),
                        )
                    nc.vector.tensor_scalar_add(
                        dst[:, c * CH + lo : c * CH + hi],
                        ps[:, 0 : hi - lo],
                        b_sb[:, 0:1],
                    )

                piece(wq_sb, bq_sb, QT, xt["q"], 0, CH)
                if c == 0:
                    # chunk 0's K projection split so key-tile 0 (the
                    # only 128 columns the first scores need) is ready
                    # one DVE-add earlier - starts the exp train sooner
                    piece(wk_sb, bk_sb, KT, xt["k"], 0, P)
                    piece(wk_sb, bk_sb, KT, xt["k"], P, CH)
                else:
                    piece(wk_sb, bk_sb, KT, xt["k"], 0, CH)

            def proj_v(c, xt):
                # V natural-layout tiles for this chunk (+ones column)
                for tl_i in range(CH // P):
                    ti = c * (CH // P) + tl_i
                    ps = psq.tile([P, P], f32, tag="psq", name="psv")
                    for k in range(NK):
                        mm(
                            ps[:],
                            xt["v"][:, k * CH + tl_i * P : k * CH + (tl_i + 1) * P],
                            wv_sb[:, k * P : (k + 1) * P],
                            start=(k == 0),
                            stop=(k == NK - 1),
                        )
                    o = ti * VSTRIDE
                    # both heads in one strided add: dst views cols
                    # [o..o+63] and [o+65..o+128] (skipping the shared
                    # ones column) as a [2, 64] free pattern
                    nc.vector.tensor_add(
                        V[:, o : o + 130].rearrange("p (a c) -> p a c", c=65)[
                            :, :, 0:64
                        ],
                        ps[:].rearrange("p (a c) -> p a c", c=64),
                        bvb_sb[:].rearrange("p (a c) -> p a c", c=64),
                    )

            def scores_block(q, t0, t1):
                qs = slice(q * 512, (q + 1) * 512)
                ess = []
                for t in range(t0, t1):
                    sps = pss.tile([P, 1024], f32, tag="sps", name="sps")
                    for h in range(2):
                        hp = 64 * h
                        mm(
                            sps[:, h * 512 : (h + 1) * 512],
                            KT[hp : hp + 64, t * P : (t + 1) * P],
                            QT[hp : hp + 64, qs],
                            start=True,
                            stop=True,
                        )
                    es = espool.tile([P, 1024], f16, tag="es", name="es")
                    nc.scalar.activation(es[:], sps[:], Exp, scale=SCALE)
                    ess.append(es)
                return ess

            def ctx_block(cps, t0, t1, ess):
                for t, es in zip(range(t0, t1), ess):
                    for h in range(2):
                        vo = t * VSTRIDE + 64 * h
                        mm(
                            cps[h][:],
                            V[:, vo : vo + 65],
                            es[:, h * 512 : (h + 1) * 512],
                            start=(t == 0),
                            stop=(t == NT - 1),
                        )

            def attn_block(q, cps, t0, t1):
                ess = scores_block(q, t0, t1)
                ctx_block(cps, t0, t1, ess)

            def evac(prev, final=False):
                # evacuate ctx^T PSUM -> SBUF.  Mid-kernel both copies
                # go to the DVE (ScalarE is the saturated engine and
                # the exp queue is pre-fed across boundaries); only the
                # final tail borrows ScalarE, which is idle by then.
                csbs = {}
                for h in range(2):
                    csbs[h] = csbpool.tile([65, 512], f32, tag="csb", name="csb")
                nc.vector.tensor_copy(csbs[0][:], prev[0][:])
                if final:
                    nc.scalar.copy(csbs[1][:], prev[1][:])
                else:
                    nc.vector.tensor_copy(csbs[1][:], prev[1][:])
                return csbs

            def tail_finish(q, csbs, final=False):
                # transpose back to natural layout, normalize, store.
                # ot collects the whole [128, 512] q-block so the store
                # is one contiguous DMA.  The final tail splits the
                # normalize muls across DVE/ScalarE (both idle by then)
                # and stripes the store over both HWDGE queues.
                ot = opool.tile([P, 512], f32, tag="ot", name="ot")
                for u in range(4):
                    for h in range(2):
                        sumcol = 64 if h == 0 else 0
                        # transposes borrow the (idle) projection-PSUM pool
                        tp = psq.tile([P, 65], f32, tag="psq", name="tp")
                        nc.tensor.transpose(
                            tp[:], csbs[h][:, u * P : (u + 1) * P], id_sb[0:65, 0:65]
                        )
                        r = opool.tile([P, 1], f32, tag="recip", bufs=4, name="r")
                        nc.vector.reciprocal(r[:], tp[:, sumcol : sumcol + 1])
                        dcol = u * P + 64 * h
                        src = tp[:, 0:64] if h == 0 else tp[:, 1:65]
                        if final and h == 1:
                            nc.scalar.mul(ot[:, dcol : dcol + 64], src, r[:, 0:1])
                        else:
                            nc.vector.tensor_scalar_mul(
                                ot[:, dcol : dcol + 64], src, r[:, 0:1]
                            )
                qs = q * 512
                if final:
                    nc.sync.dma_start(io["out"][:, qs : qs + 256], ot[:, 0:256])
                    nc.scalar.dma_start(
                        io["out"][:, qs + 256 : qs + 512], ot[:, 256:512]
                    )
                else:
                    nc.sync.dma_start(io["out"][:, qs : qs + 512], ot[:])

            def new_cps():
                return {
                    0: psc.tile([65, 512], f32, tag="c0", name="c0"),
                    1: psc.tile([65, 512], f32, tag="c1", name="c1"),
                }

            # issue all input DMAs upfront (3 queues, 4 chunks deep)
            xts = [load_chunk(c) for c in range(S // CH)]
            pe_warmup(16)
            # q0 phase, wavefront-scheduled: proj_qk(c+1) is emitted
            # before proj_v(c)/ctx(c) so the next chunk's QT/KT
            # bias-adds sit ahead of the V evacuation in the DVE FIFO,
            # and q1's first 8 score tiles (which only need QT chunk 1
            # + KT tile t) interleave into the PE-heavy q0 phase so
            # ScalarE never starves.
            cps = new_cps()
            proj_qk(0, xts[0])
            ess0 = scores_block(0, 0, 4)
            proj_qk(1, xts[1])
            ess1 = []
            for c in range(S // CH):
                proj_v(c, xts[c])
                ctx_block(cps, 4 * c, 4 * (c + 1), ess0)
                if c + 1 < S // CH:
                    ess0 = scores_block(0, 4 * (c + 1), 4 * (c + 2))
                if c + 2 < S // CH:
                    proj_qk(c + 2, xts[c + 2])
                if c < 3:
                    ess1 += scores_block(1, 4 * c, 4 * (c + 1))
            # q1: ctx over the pre-computed tiles 0-11; scores for the
            # remaining tiles and the next block's head keep ScalarE's
            # queue non-empty across every boundary
            prev, cps = cps, new_cps()
            csbs = evac(prev)
            ess1 += scores_block(1, 12, 14)
            ctx_block(cps, 0, 4, ess1[0:4])
            tail_finish(0, csbs)
            ess1 += scores_block(1, 14, NT)
            ctx_block(cps, 4, 12, ess1[4:12])
            ess2 = scores_block(2, 0, 6)
            ctx_block(cps, 12, NT, ess1[12:16])
            # q2
            prev, cps = cps, new_cps()
            csbs = evac(prev)
            ess2 += scores_block(2, 6, 8)
            ctx_block(cps, 0, 6, ess2[0:6])
            ess2 += scores_block(2, 8, 10)
            tail_finish(1, csbs)
            ctx_block(cps, 6, 10, ess2[6:10])
            ess2 += scores_block(2, 10, NT)
            ess3 = scores_block(3, 0, 6)
            ctx_block(cps, 10, NT, ess2[10:16])
            # q3
            prev, cps = cps, new_cps()
            csbs = evac(prev)
            ess3 += scores_block(3, 6, 8)
            ctx_block(cps, 0, 6, ess3[0:6])
            ess3 += scores_block(3, 8, 10)
            tail_finish(2, csbs)
            ctx_block(cps, 6, 10, ess3[6:10])
            ess3 += scores_block(3, 10, NT)
            ctx_block(cps, 10, NT, ess3[10:16])
            csbs = evac(cps, final=True)
            tail_finish(3, csbs, final=True)


def _get_program():
    if "nc" not in _CACHE:
        _CACHE["nc"] = _build_program()
    return _CACHE["nc"]


def _prearrange_xT(x, dtype):
    """[S, DIM] fp32 -> [128, NK*S] with
    [p, c*2048 + k*512 + s] = x[c*512+s, 128k+p]."""
    xT = np.ascontiguousarray(x.T.astype(dtype))  # [512, 2048]
    return np.ascontiguousarray(
        xT.reshape(NK, P, S // CH, CH).transpose(1, 2, 0, 3).reshape(P, NK * S)
    )


def _shard_inputs(query, key, value, Wq, bq, Wk, bk, Wv, bv):
    """Build the 8 per-core input dicts (q/k path fp8, v path fp16)."""
    f8 = ml_dtypes.float8_e3m4
    ident = np.eye(P, dtype=np.float32)
    maps = []
    xP = {}
    for b in range(B):
        xP[b] = (
            _prearrange_xT(query[b], f8),
            _prearrange_xT(key[b], f8),
            _prearrange_xT(value[b], np.float16),
        )

    def wslice(W, g, dtype, scale=1.0):
        # want w[p, 128k + m] = scale * W[128g + m, 128k + p]
        Ws = W[P * g : P * (g + 1), :] * scale  # [m, 512]
        return np.ascontiguousarray(
            Ws.reshape(P, NK, P).transpose(2, 1, 0).reshape(P, DIM).astype(dtype)
        )

    for c in range(N_CORES):
        b, g = c // 4, c % 4
        sl = slice(P * g, P * (g + 1))
        maps.append(
            {
                "xq": xP[b][0],
                "xk": xP[b][1],
                "xv": xP[b][2],
                "wq": wslice(Wq, g, f8, WSCALE),
                "wk": wslice(Wk, g, f8, WSCALE),
                "wv": wslice(Wv, g, np.float16),
                "bq2": np.ascontiguousarray(
                    WSCALE * bq[sl].reshape(P, 1), dtype=np.float32
                ),
                "bk2": np.ascontiguousarray(
                    WSCALE * bk[sl].reshape(P, 1), dtype=np.float32
                ),
                "bvb": np.ascontiguousarray(
                    np.broadcast_to(bv[sl], (P, P)), dtype=np.float32
                ),
                "ident": ident,
            }
        )
    return maps


def _numpy_reference(query, key, value, mask, Wq, bq, Wk, bk, Wv, bv):
    """Pure-numpy fallback (only used when the mask isn't all ones)."""
    out = np.empty((B, S, DIM), dtype=np.float32)
    for b in range(B):
        q = (query[b] @ Wq.T + bq).reshape(S, NUM_HEADS, D_HEAD)
        k = (key[b] @ Wk.T + bk).reshape(S, NUM_HEADS, D_HEAD)
        v = (value[b] @ Wv.T + bv).reshape(S, NUM_HEADS, D_HEAD)
        for h in range(NUM_HEADS):
            s = q[:, h, :] @ k[:, h, :].T
            s = np.where(mask[b], s, np.float32(-10000.0))
            s = s / np.float32(np.sqrt(DIM))
            s = s - s.max(axis=-1, keepdims=True)
            e = np.exp(s)
            p = e / e.sum(axis=-1, keepdims=True)
            out[b, :, h * D_HEAD : (h + 1) * D_HEAD] = p @ v[:, h, :]
    return out


LAST_EXEC_NS = None
LAST_RESULTS = None


def kernel(query, key, value, mask, Wq, bq, Wk, bk, Wv, bv):
    global LAST_EXEC_NS, LAST_RESULTS
    query = np.asarray(query, dtype=np.float32)
    key = np.asarray(key, dtype=np.float32)
    value = np.asarray(value, dtype=np.float32)
    mask = np.asarray(mask)
    Wq = np.asarray(Wq, dtype=np.float32)
    bq = np.asarray(bq, dtype=np.float32)
    Wk = np.asarray(Wk, dtype=np.float32)
    bk = np.asarray(bk, dtype=np.float32)
    Wv = np.asarray(Wv, dtype=np.float32)
    bv = np.asarray(bv, dtype=np.float32)

    if not mask.all():
        return _numpy_reference(query, key, value, mask, Wq, bq, Wk, bk, Wv, bv)

    from concourse.bass_utils import run_bass_kernel_spmd

    nc = _get_program()
    in_maps = _shard_inputs(query, key, value, Wq, bq, Wk, bk, Wv, bv)
    trace = os.environ.get("KERNEL_TRACE", "0") == "1"
    tmpdir = os.environ.get("KERNEL_TRACE_DIR") or None
    try:
        res = run_bass_kernel_spmd(
            nc, in_maps, list(range(N_CORES)), trace=trace, tmpdir=tmpdir
        )
    except Exception:
        if not trace:
            raise
        import traceback

        traceback.print_exc()
        res = run_bass_kernel_spmd(nc, in_maps, list(range(N_CORES)), trace=False)
    LAST_EXEC_NS = res.exec_time_ns
    LAST_RESULTS = res
    out = np.empty((B, S, DIM), dtype=np.float32)
    for c in range(N_CORES):
        b, g = c // 4, c % 4
        # untile [128, 16*128] -> [2048, 128]
        o = np.asarray(res.results[c]["out"]).reshape(P, NT, P).transpose(1, 0, 2)
        out[b, :, P * g : P * (g + 1)] = o.reshape(S, P)
    return out

